# revision 21
# baseline (speedup 1.0000x reference)
"""Trainium2 Bass kernel for nn_PonitaFC (PONITA fully-connected GNN).

M=B*N super-nodes sharded data-parallel across 8 NeuronCores (32 each).
Host: knn index selection + gathers, geometric invariants -> 9 poly
monomials, weight folds.  Device: basis MLP -> kb, kern = kb @ Wk,
K-neighbor einsum (DVE), fiber conv as block-diagonal PE matmuls over
(o, c6), LayerNorm via PE transposes, bottleneck MLP, residual updates,
readout + pooling.  kb spilled to HBM in bf16 between layers.

Hardcodes B=1, N=256, D=3, K=8, O=20, H=BD=128, L=2, OUT=OV=16, 8 cores.
"""

import contextlib
import os
import numpy as np

import concourse.bass as bass
import concourse.tile as tile
from concourse import mybir
from concourse.bass_utils import run_bass_kernel_spmd
from concourse.vector_clock import ScopedClock

N_CORES = 8
B, N, D = 1, 256, 3
K = 8
O = 20
H = 128
BD = 128
L = 2
OUT = 16
OV = 16
M = B * N

MC = M // N_CORES          # 32 super-nodes per core
MCH = 4                    # super-nodes per phase-A chunk
NCH = MC // MCH            # 8 chunks
RCH = MCH * O * K * K      # 5120 kb-rows per chunk
HCH = MCH * O * K          # 640 h-rows per chunk
RH = MC * O * K            # 5120 h-rows per core
RKB = MC * O * K * K       # 40960 kb-rows per core

F32 = mybir.dt.float32
F32R = mybir.dt.float32r
BF16 = mybir.dt.bfloat16
AF = mybir.ActivationFunctionType
ALU = mybir.AluOpType
AX = mybir.AxisListType

DEBUG_DUMPS = bool(int(os.environ.get("KERNEL_DEBUG_DUMPS", "0")))


class PatchedTileContext(tile.TileContext):
    """This container's walrus rejects >1 sem wait per instruction; split the
    final drain's waits onto preceding single-wait NOPs."""
    N_SPARE = 30

    def _drain_and_barrier(self, tick_clock, wait_clock):
        nc = self.nc
        nops = [nc.sync.nop(nofuse=True) for _ in range(self.N_SPARE)]
        drain_inst = nc.sync.drain()
        wait_clock.add_sem_waits(
            drain_inst.ins, ScopedClock({None: tick_clock.global_clock})
        )
        si = drain_inst.ins.sync_info
        waits = list(si.on_wait) if si is not None else []
        if len(waits) > 1:
            assert len(waits) - 1 <= len(nops), f"drain waits: {len(waits)}"
            for i, wt in enumerate(waits[:-1]):
                nops[i].ins.sync_info = mybir.SyncInfo(on_wait=[wt],
                                                       on_update=[])
            drain_inst.ins.sync_info = mybir.SyncInfo(
                on_wait=[waits[-1]],
                on_update=list(si.on_update) if si is not None else [],
            )
        nc.all_engine_barrier()
        popped = nc._tile_sem_poison_stack.pop()
        assert popped is self._sem_poison
        nc.clear_and_free_semaphores(list(self.sems.allocated().values()))
        nc.all_engine_barrier()


# ----------------------------------------------------------------------------
# host-side math
# ----------------------------------------------------------------------------
def _ori_grid(n=O):
    i = np.arange(n, dtype=np.float64)
    theta = (np.pi * i * (1.0 + np.sqrt(5.0))) % (2.0 * np.pi)
    phi = np.arccos(1.0 - 2.0 * (i + 0.5) / (n - 1 + 1.0))
    g = np.stack([np.cos(theta) * np.sin(phi),
                  np.sin(theta) * np.sin(phi),
                  np.cos(phi)], axis=-1)
    return g.astype(np.float32)


def _knn_idx(x):
    xb = x[0].astype(np.float32)
    diff = xb[:, None, :] - xb[None, :, :]
    sq = (diff * diff).astype(np.float32)
    d2 = ((sq[..., 0] + sq[..., 1]) + sq[..., 2]).astype(np.float32)
    return np.argsort(d2, axis=-1, kind="stable")[:, :K]


_C9 = np.zeros((9, 14), dtype=np.float32)
for _j, _fs in enumerate([[0], [1], [2], [3, 4], [5], [6], [7, 8, 10],
                          [9, 11, 12], [13]]):
    for _f in _fs:
        _C9[_j, _f] = 1.0


def _mono9(x, y):
    x2 = x * x
    y2 = y * y
    xy = x * y
    return np.stack([x, y, x2, xy, y2, x2 * x, x2 * y, xy * y, y2 * y], axis=0)


def _host_prep(inputs):
    f32 = np.float32
    g = {k: np.asarray(v, f32) for k, v in inputs.items()}
    ori = _ori_grid()

    idx = _knn_idx(g["x"])                                # [N,K]
    xn = g["x"][0][idx]                                   # [N,K,3]
    pn = g["pos"][0][idx]                                 # [N,K,3]
    rel = pn[:, None, :, :] - pn[:, :, None, :]           # [M,a,b,3]
    relsq = (rel * rel).astype(f32)
    relsq = ((relsq[..., 0] + relsq[..., 1]) + relsq[..., 2]).astype(f32)

    rel_s = rel.transpose(3, 0, 1, 2).reshape(3, M * K * K)   # [3,(m,a,b)]
    inv1 = (ori @ rel_s).astype(f32)                      # [O,(m,a,b)]
    t = (1.0 - inv1).astype(f32)
    i2 = ((t * t) * relsq.reshape(1, -1)).astype(f32)
    inv2 = np.sqrt(i2 + np.float32(1e-12)).astype(f32)
    monos = _mono9(inv1, inv2)                            # [9,O,(m,a,b)]
    monos = (monos.reshape(9, O, M, K, K)                 # j,o,m,a,b
             .transpose(0, 2, 1, 4, 3)                    # j,m,o,b,a
             .reshape(9, M * O * K * K)).astype(f32)

    a9 = (_C9 @ g["W1"]).astype(f32)                      # [9,H]
    Wfk_s = (g["Wfk"] / f32(H)).astype(f32)               # [L,BD,H]
    Wb1p = np.stack([g["gamma"][l][:, None] * g["Wb1"][l] for l in range(L)])
    bias1p = np.stack([g["beta"][l] @ g["Wb1"][l] + g["bb1"][l]
                       for l in range(L)])                # [L,4H]
    Wr_s = (g["Wr"] / f32(L * K * O)).astype(f32)         # [L,H,32]

    oi = (ori @ ori.T).astype(f32)
    mono_oi = np.stack([oi, oi * oi, (oi * oi) * oi], 0).reshape(3, O * O)

    xn_cm = xn.reshape(M * K, 3).T.astype(f32)            # [3,(m,s)]

    # cbias per block: cb[(ct*20+o), l*22+j] = cbias[l, 6j+ct]
    cb = np.zeros((120, L * 22), f32)
    for l in range(L):
        for j in range(22):
            nch = min(6, H - 6 * j)
            for ct in range(nch):
                cb[ct * 20 + np.arange(O), l * 22 + j] = \
                    g["cbias"][l, 6 * j + ct]

    # Wb2 as [128, 4, 128] k-tiles
    wb2 = [np.ascontiguousarray(
        g["Wb2"][l].reshape(4, 128, 128).transpose(1, 0, 2)) for l in range(L)]

    shared = dict(
        a9=a9, b1=g["b1"].reshape(H, 1), W2=g["W2"], b2=g["b2"].reshape(H, 1),
        We=g["We"], af=g["Wf1"], bf1=g["bf1"].reshape(H, 1), Wf2=g["Wf2"],
        bf2=g["bf2"].reshape(H, 1), monoi=mono_oi,
        Wk0=g["Wk"][0], Wk1=g["Wk"][1],
        Wb1p0=Wb1p[0], Wb1p1=Wb1p[1],
        bias1p0=np.ascontiguousarray(bias1p[0].reshape(4, H).T),  # [128,4]
        bias1p1=np.ascontiguousarray(bias1p[1].reshape(4, H).T),
        Wb2_0=wb2[0], Wb2_1=wb2[1],
        bb2_0=g["bb2"][0].reshape(H, 1), bb2_1=g["bb2"][1].reshape(H, 1),
        Wr0=Wr_s[0], Wr1=Wr_s[1],
        Wfk0=Wfk_s[0], Wfk1=Wfk_s[1],
        cb=cb, oriT=ori, ident=np.eye(128, dtype=f32),
    )

    per_core = []
    monos_pc = monos.reshape(9, M, O * K * K)
    xn_pc = xn_cm.reshape(3, M, K)
    for c in range(N_CORES):
        ms = slice(c * MC, (c + 1) * MC)
        per_core.append(dict(
            monos=np.ascontiguousarray(monos_pc[:, ms].reshape(9, RKB)),
            xn=np.ascontiguousarray(xn_pc[:, ms].reshape(3, MC * K)),
        ))
    return shared, per_core, ori, g


# ----------------------------------------------------------------------------
# device program
# ----------------------------------------------------------------------------
def _r(ap):
    return ap.bitcast(F32R)


_DW_SHAPES = [
    ("a9", [9, H]), ("b1", [H, 1]), ("W2", [H, BD]), ("b2", [H, 1]),
    ("We", [3, H]), ("af", [3, H]), ("bf1", [H, 1]), ("Wf2", [H, BD]),
    ("bf2", [H, 1]), ("monoi", [3, O * O]),
    ("Wk0", [BD, H]), ("Wk1", [BD, H]),
    ("Wb1p0", [H, 4 * H]), ("Wb1p1", [H, 4 * H]),
    ("bias1p0", [H, 4]), ("bias1p1", [H, 4]),
    ("Wb2_0", [128, 4, 128]), ("Wb2_1", [128, 4, 128]),
    ("bb2_0", [H, 1]), ("bb2_1", [H, 1]),
    ("Wr0", [H, 32]), ("Wr1", [H, 32]),
    ("Wfk0", [BD, H]), ("Wfk1", [BD, H]),
    ("cb", [120, L * 22]), ("oriT", [O, 3]), ("ident", [128, 128]),
]


_F32R_W = {"a9", "W2", "We", "af", "Wf2", "monoi", "Wb1p0", "Wb1p1",
           "Wr0", "Wr1", "Wfk0", "Wfk1", "oriT", "ident"}


def _build_program():
    nc = bass.Bass()

    d_monos = nc.dram_tensor("monos", [9, RKB], F32R, kind="ExternalInput")
    d_xn = nc.dram_tensor("xn", [3, MC * K], F32R, kind="ExternalInput")
    dw = {}
    for name, shape in _DW_SHAPES:
        dt_ = F32R if name in _F32R_W else F32
        dw[name] = nc.dram_tensor(name, shape, dt_, kind="ExternalInput")

    d_scal = nc.dram_tensor("scal", [OUT, MC], F32, kind="ExternalOutput")
    d_vec = nc.dram_tensor("vec", [3, MC * OV], F32, kind="ExternalOutput")
    d_spill = nc.dram_tensor("kb_spill", [128, RKB], BF16, kind="Internal")

    dbg = {}
    if DEBUG_DUMPS:
        for name, shape, dt_ in [
            ("dbg_kb", [128, RCH], BF16), ("dbg_kern", [128, RCH], BF16),
            ("dbg_x1_0", [128, RH], BF16), ("dbg_x2s_0", [128, RH], BF16),
            ("dbg_z_0", [128, RH], F32), ("dbg_h1", [128, RH], F32),
            ("dbg_rsum", [32, RH], F32), ("dbg_hL0", [128, RH], F32),
            ("dbg_x1_1", [128, RH], BF16), ("dbg_x2s_1", [128, RH], BF16),
            ("dbg_z_1", [128, RH], F32), ("dbg_y1", [128, 4 * 1024], BF16),
        ]:
            dbg[name] = nc.dram_tensor(name, shape, dt_,
                                       kind="ExternalOutput")

    import os as _os
    _lin = bool(int(_os.environ.get("KERNEL_LINEARIZE", "0")))
    with PatchedTileContext(nc, linearize=_lin) as tc:
        _emit(tc, nc, d_monos, d_xn, dw, d_scal, d_vec, d_spill, dbg)
    _install_multiwait_split(nc)
    return nc


def _split_multiwait_json(data):
    """Walrus in this container rejects >1 sem wait per instruction; move
    extra waits onto preceding same-engine NoOps at the BIR JSON level."""
    import json
    d = json.loads(data)
    n_split = 0
    for fn in d["functions"]:
        for blk in fn["blocks"]:
            out = []
            for inst in blk["instructions"]:
                si = inst.get("sync_info")
                waits = (si or {}).get("on_wait") or []
                if len(waits) > 1:
                    n_split += 1
                    for i, wt in enumerate(waits[:-1]):
                        out.append({
                            "name": inst["name"] + f"__w{i}",
                            "opcode": "NoOp",
                            "engine": inst["engine"],
                            "ins": [], "outs": [],
                            "sync_info": {"on_wait": [wt], "on_update": []},
                        })
                    si["on_wait"] = [waits[-1]]
                out.append(inst)
            blk["instructions"] = out
    return json.dumps(d).encode()


def _install_multiwait_split(nc):
    orig = nc.to_json_bytes

    def patched(*a, **k):
        return _split_multiwait_json(orig(*a, **k))

    nc.to_json_bytes = patched


def _emit(tc, nc, d_monos, d_xn, dw, d_scal, d_vec, d_spill, dbg):
    ctx = contextlib.ExitStack()
    with ctx:
        wpool = ctx.enter_context(tc.tile_pool(name="weights", bufs=1))
        live = ctx.enter_context(tc.tile_pool(name="live", bufs=1))

        w = {}
        for name in ["a9", "b1", "W2", "b2", "We", "af", "bf1", "Wf2", "bf2",
                     "monoi", "Wb1p0", "Wb1p1", "bias1p0", "bias1p1",
                     "bb2_0", "bb2_1", "Wr0", "Wr1", "Wfk0", "Wfk1",
                     "cb", "oriT", "ident"]:
            t = wpool.tile(list(dw[name].shape),
                           F32R if name in _F32R_W else F32, tag=name)
            nc.sync.dma_start(t[:], dw[name][:])
            w[name] = t
        wb = {}
        for name in ["Wk0", "Wk1", "Wb2_0", "Wb2_1"]:
            tf = wpool.tile(list(dw[name].shape), F32, tag=name + "_f")
            nc.sync.dma_start(tf[:], dw[name][:])
            t = wpool.tile(list(dw[name].shape), BF16, tag=name)
            nc.vector.tensor_copy(t[:], tf[:])
            wb[name] = t
        ones_b = wpool.tile([128, 1], BF16, tag="ones_b")
        nc.vector.memset(ones_b[:], 1.0)
        ones_f = wpool.tile([128, 1], F32, tag="ones_f")
        nc.vector.memset(ones_f[:], 1.0)
        eps_t = wpool.tile([128, 1], F32, tag="eps_t")
        nc.vector.memset(eps_t[:], 1e-5)
        ident_b = wpool.tile([128, 128], BF16, tag="ident_b")
        nc.vector.tensor_copy(ident_b[:], w["ident"][:])
        cst = dict(ones_b=ones_b, ones_f=ones_f, eps_t=eps_t,
                   ident_b=ident_b)

        h = live.tile([128, RH], F32R, tag="h")             # rows (o,m,s)
        x1 = live.tile([128, RH], BF16, tag="x1")           # rows (o,m,a)
        rsum = live.tile([32, RH], F32, tag="rsum")         # rows (p,m,a)
        h0emb = live.tile([128, MC * K], F32R, tag="h0emb")  # (m,s)
        xn_t = live.tile([3, MC * K], F32R, tag="xn")
        nc.sync.dma_start(xn_t[:], d_xn[:])
        fkb = [live.tile([120, 22, 120], BF16, tag=f"fkb{l}",
                         name=f"fkb{l}") for l in range(L)]

        # ---- setup: h0emb + fiber basis ----
        with tc.tile_pool(name="ps_set", bufs=2, space="PSUM") as pss, \
             tc.tile_pool(name="sb_set", bufs=2) as sbs:
            pt = pss.tile([128, MC * K], F32, tag="pA")
            nc.tensor.matmul(pt[:], (w["We"][:]), (xn_t[:]),
                             start=True, stop=True)
            nc.scalar.activation(h0emb[:], pt[:], AF.Copy)
            # materialize h0 broadcast over fiber: h[c,(o,m,s)] = h0emb[c,(m,s)]
            hdst = h[:].rearrange("c (o ms) -> c o ms", o=O)
            hsrc = bass.AP(tensor=h0emb.tensor, offset=h0emb.offset,
                           ap=[list(h0emb.ap[0]), [0, O], [1, MC * K]])
            nc.sync.dma_start(hdst, hsrc)

            uf = pss.tile([128, O * O], F32, tag="pA")
            nc.tensor.matmul(uf[:], (w["af"][:]), (w["monoi"][:]),
                             start=True, stop=True)
            s1f = sbs.tile([128, O * O], F32R, tag="s1f")
            nc.scalar.activation(s1f[:], uf[:], AF.Gelu, bias=w["bf1"][:])
            kbf = pss.tile([128, O * O], F32, tag="pA")
            nc.tensor.matmul(kbf[:], (w["Wf2"][:]), _r(s1f[:]),
                             start=True, stop=True)
            fbv = sbs.tile([128, O * O], F32R, tag="fbv")
            nc.scalar.activation(fbv[:], kbf[:], AF.Gelu, bias=w["bf2"][:])
            for l in range(L):
                fkp = pss.tile([128, O * O], F32, tag="pA")
                nc.tensor.matmul(fkp[:], (w[f"Wfk{l}"][:]), _r(fbv[:]),
                                 start=True, stop=True)
                fk_s = sbs.tile([128, O * O], BF16, tag="fk_s")
                nc.vector.tensor_copy(fk_s[:], fkp[:])
                nc.vector.memset(fkb[l][:], 0.0)
                # scatter fk[6j+ct,(p,o)] -> fkb[(ct*20+o), j, (ct*20+p)]
                # fk is symmetric in (p,o), so read rows as (o,p) directly.
                for ct in range(6):
                    nblk = 22 if ct < 2 else 21
                    for j in range(nblk):
                        dst = fkb[l][ct * O:(ct + 1) * O, j,
                                     ct * O:(ct + 1) * O]     # (o, p)
                        src = fk_s[6 * j + ct:6 * j + ct + 1, :].rearrange(
                            "c (o p) -> c o p", o=O)
                        nc.sync.dma_start(dst, src)

        # ---- layers ----
        for l in range(L):
            _phase_a(tc, nc, l, d_monos, d_spill, w, wb, h, h0emb, x1, dbg)
            if DEBUG_DUMPS and l == 1:
                nc.sync.dma_start(dbg["dbg_x1_1"][:], x1[:])
            _phase_b(tc, nc, l, w, wb, cst, fkb[l], h, h0emb, x1, rsum, dbg)
            if DEBUG_DUMPS and l == 0:
                nc.sync.dma_start(dbg["dbg_hL0"][:], h[:].bitcast(F32))

        # ---- pooling ----
        with tc.tile_pool(name="ps_pool", bufs=2, space="PSUM") as psp, \
             tc.tile_pool(name="sb_pool", bufs=2) as sbp:
            # step 1: reduce slots -> pa [32, (p, m)]
            pa = sbp.tile([32, O * MC], F32R, tag="pa")
            rs_pm = rsum[:, :].rearrange("ch (pm g) -> ch pm g", g=K)
            with nc.allow_low_precision(reason="f32r is full fp32 storage"):
                nc.vector.tensor_reduce(pa[:], rs_pm, AX.X, ALU.add)
            # step 2 (scaler): reduce over p with (m, p) iteration
            scal = sbp.tile([OUT, MC], F32, tag="scal")
            pa_sc = bass.AP(tensor=pa.tensor, offset=pa.offset,
                            ap=[[pa.ap[0][0], OUT], [1, MC], [MC, O]])
            nc.vector.tensor_reduce(scal[:], pa_sc, AX.X, ALU.add)
            nc.sync.dma_start(d_scal[:], scal[:])

            # vat [20, (v, m)]
            vat = sbp.tile([O, OV * MC], F32R, tag="vat")
            for v in range(OV):
                dst = vat[:, v * MC:(v + 1) * MC]
                src = pa[OUT + v:OUT + v + 1, :].rearrange(
                    "c (p m) -> c p m", p=O)
                nc.sync.dma_start(dst, src)
            pv = psp.tile([3, MC * OV], F32, tag="pv")
            nc.tensor.matmul(pv[:], (w["oriT"][:]), (vat[:]),
                             start=True, stop=True)
            vec_o = sbp.tile([3, MC * OV], F32, tag="vec_o")
            nc.scalar.activation(vec_o[:], pv[:], AF.Copy)
            nc.sync.dma_start(d_vec[:], vec_o[:])

        if DEBUG_DUMPS:
            nc.sync.dma_start(dbg["dbg_rsum"][:], rsum[:])
            nc.sync.dma_start(dbg["dbg_h1"][:], h[:].bitcast(F32))


def _phase_a(tc, nc, l, d_monos, d_spill, w, wb, h, h0emb, x1, dbg):
    """Per chunk: (l==0) monos -> mm1 -> gelu -> mm2 -> gelu -> kb bf16
    (spilled to HBM); (l==1) reload kb.  Then kern = kb @ Wk_l and the
    K-neighbor einsum accumulating x1."""
    SUB = 1024
    NSUB = RCH // SUB  # 5
    with tc.tile_pool(name="apool", bufs=2) as apool, \
         tc.tile_pool(name="apool1", bufs=1) as apool1, \
         tc.tile_pool(name="pwpool", bufs=2) as pwpool, \
         tc.tile_pool(name="ps_mm", bufs=3, space="PSUM") as psmm, \
         tc.tile_pool(name="ps_kern", bufs=2, space="PSUM") as pskern:
        for ch in range(NCH):
            r0 = ch * RCH
            kbt = apool.tile([128, RCH], BF16, tag="kbt")
            if l == 0:
                mt = apool1.tile([9, RCH], F32R, tag="mt")
                nc.sync.dma_start(mt[:], d_monos[:, r0:r0 + RCH])
                for sb in range(NSUB):
                    s0 = sb * SUB
                    u_ps = psmm.tile([128, SUB], F32, tag="mmps")
                    for q in range(2):
                        nc.tensor.matmul(
                            u_ps[:, q * 512:(q + 1) * 512],
                            (w["a9"][:]),
                            (mt[:, s0 + q * 512:s0 + (q + 1) * 512]),
                            start=True, stop=True)
                    s1t = apool.tile([128, SUB], F32R, tag="s1t")
                    nc.scalar.activation(s1t[:], u_ps[:], AF.Gelu,
                                         bias=w["b1"][:])
                    k_ps = psmm.tile([128, SUB], F32, tag="mmps")
                    for q in range(2):
                        nc.tensor.matmul(
                            k_ps[:, q * 512:(q + 1) * 512],
                            (w["W2"][:]),
                            (s1t[:, q * 512:(q + 1) * 512]),
                            start=True, stop=True)
                    nc.scalar.activation(kbt[:, s0:s0 + SUB], k_ps[:],
                                         AF.Gelu, bias=w["b2"][:])
                nc.sync.dma_start(d_spill[:, r0:r0 + RCH], kbt[:])
            else:
                nc.sync.dma_start(kbt[:], d_spill[:, r0:r0 + RCH])

            kernt = apool.tile([128, RCH], BF16, tag="kernt")
            for sb in range(RCH // 512):
                s0 = sb * 512
                kp = pskern.tile([128, 512], F32, tag="kernps")
                nc.tensor.matmul(kp[:], wb[f"Wk{l}"][:],
                                 kbt[:, s0:s0 + 512], start=True, stop=True)
                nc.vector.tensor_copy(kernt[:, s0:s0 + 512], kp[:])

            if DEBUG_DUMPS and l == 0 and ch == 0:
                nc.sync.dma_start(dbg["dbg_kb"][:], kbt[:])
                nc.sync.dma_start(dbg["dbg_kern"][:], kernt[:])

            # x1[c,(o,m,a)] = sum_b kern[c,(m,o,b,a)] * h[c,(o,m,b)]
            kern5 = kernt[:].rearrange("c (m o b a) -> c m o b a",
                                       m=MCH, o=O, b=K)
            # h view (m, o, s): h rows are (o, m, s) core-wide
            h4 = h[:].rearrange("c (o m s) -> c o m s", o=O, m=MC)
            pw = []
            for b in range(K):
                pwt = pwpool.tile([128, HCH], BF16, tag=f"pw{b}")
                hsl = bass.AP(
                    tensor=h.tensor, offset=h.offset + b + ch * MCH * K,
                    ap=[list(h.ap[0]), [K, MCH], [MC * K, O], [0, K]])
                pwv = pwt[:].rearrange("c (m o a) -> c m o a", m=MCH, o=O)
                nc.vector.tensor_tensor(pwv, kern5[:, :, :, b, :], hsl,
                                        ALU.mult)
                pw.append(pwt)
            nc.vector.tensor_tensor(pw[0][:], pw[0][:], pw[1][:], ALU.add)
            nc.vector.tensor_tensor(pw[2][:], pw[2][:], pw[3][:], ALU.add)
            nc.vector.tensor_tensor(pw[4][:], pw[4][:], pw[5][:], ALU.add)
            nc.vector.tensor_tensor(pw[6][:], pw[6][:], pw[7][:], ALU.add)
            nc.vector.tensor_tensor(pw[0][:], pw[0][:], pw[2][:], ALU.add)
            nc.vector.tensor_tensor(pw[4][:], pw[4][:], pw[6][:], ALU.add)
            x1o = bass.AP(
                tensor=x1.tensor, offset=x1.offset + ch * MCH * K,
                ap=[list(x1.ap[0]), [K, MCH], [MC * K, O], [1, K]])
            pwv0 = pw[0][:].rearrange("c (m o a) -> c m o a", m=MCH, o=O)
            pwv4 = pw[4][:].rearrange("c (m o a) -> c m o a", m=MCH, o=O)
            nc.vector.tensor_tensor(x1o, pwv0, pwv4, ALU.add)


def _phase_b(tc, nc, l, w, wb, cst, fkb_l, h, h0emb, x1, rsum, dbg):
    """Fiber conv (block-diag PE), LayerNorm (PE transposes + deferred
    per-row scalars), bottleneck MLP, residual update, readout."""
    with tc.tile_pool(name="bpool", bufs=1) as bpool, \
         tc.tile_pool(name="bpool2", bufs=2) as bpool2:
        # ---- shuffle x1 -> x1J blocks ----
        x1j = bpool.tile([120, 22, MC * K], BF16, tag="x1j")
        for ct in range(6):
            nblk = 22 if ct < 2 else 21
            for j in range(nblk):
                dst = x1j[ct * O:(ct + 1) * O, j, :]
                src = x1[6 * j + ct:6 * j + ct + 1, :].rearrange(
                    "c (o ma) -> c o ma", o=O)
                nc.sync.dma_start(dst, src)

        # ---- fiber conv ----
        x2e = bpool.tile([120, 22, MC * K], BF16, tag="x2e")
        with tc.tile_pool(name="ps_fib", bufs=2, space="PSUM") as psf:
            for j in range(22):
                fp = psf.tile([120, MC * K], F32, tag="fib")
                nc.tensor.matmul(fp[:], fkb_l[:, j, :], x1j[:, j, :],
                                 start=True, stop=True)
                nc.vector.tensor_scalar_add(
                    x2e[:, j, :], fp[:],
                    w["cb"][:, l * 22 + j:l * 22 + j + 1])

        # ---- shuffle back -> x2s [c,(m,p,a)] ----
        x2s = bpool.tile([128, RH], BF16, tag="x2s")
        for ct in range(6):
            nblk = 22 if ct < 2 else 21
            for j in range(nblk):
                dst = x2s[6 * j + ct:6 * j + ct + 1, :].rearrange(
                    "c (p ma) -> c p ma", p=O)
                src = x2e[ct * O:(ct + 1) * O, j, :]
                nc.sync.dma_start(dst, src)

        if DEBUG_DUMPS and l == 0:
            nc.sync.dma_start(dbg["dbg_x1_0"][:], x1[:])
        if DEBUG_DUMPS:
            nc.sync.dma_start(dbg[f"dbg_x2s_{l}"][:], x2s[:])

        # ---- LayerNorm -> z [c,(m,p,a)] f32 ----
        z = bpool.tile([128, RH], F32R, tag="z")
        with tc.tile_pool(name="ps_ln", bufs=2, space="PSUM") as psl, \
             tc.tile_pool(name="sb_ln", bufs=2) as sbl:
            for g512 in range(RH // 512):
                s0 = g512 * 512
                x2sq = sbl.tile([128, 512], F32, tag="x2sq")
                nc.vector.tensor_tensor(x2sq[:], x2s[:, s0:s0 + 512],
                                        x2s[:, s0:s0 + 512], ALU.mult)
                s1p = psl.tile([128, 4], F32, tag="s1p")
                s2p = psl.tile([128, 4], F32, tag="s2p")
                for gi in range(4):
                    c0 = s0 + gi * 128
                    nc.tensor.matmul(s1p[:, gi:gi + 1],
                                     x2s[:, c0:c0 + 128], cst["ones_b"][:],
                                     start=True, stop=True)
                    nc.tensor.matmul(s2p[:, gi:gi + 1],
                                     (x2sq[:, gi * 128:(gi + 1) * 128]),
                                     (cst["ones_f"][:]),
                                     start=True, stop=True)
                muT = sbl.tile([128, 4], F32, tag="muT")
                nc.vector.tensor_scalar_mul(muT[:], s1p[:], 1.0 / 128.0)
                musq = sbl.tile([128, 4], F32, tag="musq")
                nc.vector.tensor_tensor(musq[:], muT[:], muT[:], ALU.mult)
                varT = sbl.tile([128, 4], F32, tag="varT")
                nc.vector.scalar_tensor_tensor(
                    out=varT[:], in0=s2p[:], scalar=1.0 / 128.0,
                    in1=musq[:], op0=ALU.mult, op1=ALU.subtract)
                sdT = sbl.tile([128, 4], F32, tag="sdT")
                nc.scalar.activation(sdT[:], varT[:], AF.Sqrt,
                                     bias=cst["eps_t"][:])
                rstdT = sbl.tile([128, 4], F32, tag="rstdT")
                nc.vector.reciprocal(rstdT[:], sdT[:])
                for gi in range(4):
                    c0 = s0 + gi * 128
                    xtp = psl.tile([128, 128], BF16, tag="xtp")
                    nc.tensor.transpose(xtp[:], x2s[:, c0:c0 + 128],
                                        cst["ident_b"][:])
                    zT = sbl.tile([128, 128], F32R, tag="zT")
                    nc.vector.tensor_scalar(
                        out=zT[:], in0=xtp[:],
                        scalar1=muT[:, gi:gi + 1],
                        scalar2=rstdT[:, gi:gi + 1],
                        op0=ALU.subtract, op1=ALU.mult)
                    zbp = psl.tile([128, 128], F32R, tag="zbp")
                    nc.tensor.transpose(zbp[:], zT[:], w["ident"][:])
                    nc.vector.tensor_copy(z[:, c0:c0 + 128], zbp[:])

        if DEBUG_DUMPS:
            nc.sync.dma_start(dbg[f"dbg_z_{l}"][:], z[:].bitcast(F32))

        # ---- bottleneck + h update + readout ----
        RC = 1024
        with tc.tile_pool(name="ps_bt", bufs=2, space="PSUM") as psb, \
             tc.tile_pool(name="ps_ro", bufs=2, space="PSUM") as psr:
            for rc in range(RH // RC):
                r0 = rc * RC
                y1 = bpool2.tile([128, 4, RC], BF16, tag="y1")
                for j in range(4):
                    yp = psb.tile([128, RC], F32, tag="y1p")
                    for q in range(2):
                        nc.tensor.matmul(
                            yp[:, q * 512:(q + 1) * 512],
                            (w[f"Wb1p{l}"][:, j * 128:(j + 1) * 128]),
                            (z[:, r0 + q * 512:r0 + (q + 1) * 512]),
                            start=True, stop=True)
                    nc.scalar.activation(
                        y1[:, j, :], yp[:], AF.Gelu,
                        bias=w[f"bias1p{l}"][:, j:j + 1])
                if DEBUG_DUMPS and l == 0 and rc == 0:
                    nc.sync.dma_start(dbg["dbg_y1"][:],
                                      y1[:].rearrange("c j r -> c (j r)"))
                for q in range(2):
                    q0 = r0 + q * 512
                    yo = psb.tile([128, 512], F32, tag="yo")
                    for kt in range(4):
                        nc.tensor.matmul(
                            yo[:], wb[f"Wb2_{l}"][:, kt, :],
                            y1[:, kt, q * 512:(q + 1) * 512],
                            start=(kt == 0), stop=(kt == 3))
                    nc.vector.scalar_tensor_tensor(
                        out=h[:, q0:q0 + 512], in0=yo[:],
                        scalar=w[f"bb2_{l}"][:],
                        in1=h[:, q0:q0 + 512],
                        op0=ALU.add, op1=ALU.add)
                    ro = psr.tile([32, 512], F32, tag="ro")
                    nc.tensor.matmul(ro[:], (w[f"Wr{l}"][:]),
                                     (h[:, q0:q0 + 512]),
                                     start=True, stop=True)
                    if l == 0:
                        nc.vector.tensor_copy(rsum[:, q0:q0 + 512], ro[:])
                    else:
                        nc.vector.tensor_tensor(rsum[:, q0:q0 + 512],
                                                rsum[:, q0:q0 + 512],
                                                ro[:], ALU.add)


# ----------------------------------------------------------------------------
# public entry
# ----------------------------------------------------------------------------
_CACHED = {}

_SHARED_KEYS = [name for name, _ in _DW_SHAPES]


def kernel(**inputs):
    shared, per_core, ori, g = _host_prep(inputs)
    if "nc" not in _CACHED:
        _CACHED["nc"] = _build_program()
    nc = _CACHED["nc"]

    shared_arrs = {kname: np.ascontiguousarray(shared[kname],
                                               dtype=np.float32)
                   for kname in _SHARED_KEYS}
    in_maps = []
    for c in range(N_CORES):
        mmap = dict(shared_arrs)
        mmap["monos"] = per_core[c]["monos"]
        mmap["xn"] = per_core[c]["xn"]
        in_maps.append(mmap)

    res = run_bass_kernel_spmd(nc, in_maps, core_ids=list(range(N_CORES)))

    out_scaler = np.zeros((M, OUT), np.float32)
    out_vector = np.zeros((M, OV, 3), np.float32)
    br = g["br"]
    s_off = br[:, :OUT].sum(0) / np.float32(L)
    v_off = (br[:, OUT:].sum(0))[:, None] * ori.sum(0)[None, :] / np.float32(L)
    for c in range(N_CORES):
        r = res.results[c]
        out_scaler[c * MC:(c + 1) * MC] = r["scal"].T + s_off
        out_vector[c * MC:(c + 1) * MC] = (
            r["vec"].reshape(3, OV, MC).transpose(2, 1, 0) + v_off[None])
    return out_scaler, out_vector


# revision 23
# speedup vs baseline: 1.7737x; 1.7737x over previous
"""Trainium2 Bass kernel for nn_PonitaFC (PONITA fully-connected GNN).

M=B*N super-nodes sharded data-parallel across 8 NeuronCores (32 each).
Host: knn index selection + gathers, geometric invariants -> 9 poly
monomials, weight folds.  Device: basis MLP -> kb, kern = kb @ Wk,
K-neighbor einsum (DVE), fiber conv as block-diagonal PE matmuls over
(o, c6), LayerNorm via PE transposes, bottleneck MLP, residual updates,
readout + pooling.  kb spilled to HBM in bf16 between layers.

Hardcodes B=1, N=256, D=3, K=8, O=20, H=BD=128, L=2, OUT=OV=16, 8 cores.
"""

import contextlib
import os
import numpy as np

import concourse.bass as bass
import concourse.tile as tile
from concourse import mybir
from concourse.bass_utils import run_bass_kernel_spmd
from concourse.vector_clock import ScopedClock

N_CORES = 8
B, N, D = 1, 256, 3
K = 8
O = 20
H = 128
BD = 128
L = 2
OUT = 16
OV = 16
M = B * N

MC = M // N_CORES          # 32 super-nodes per core
MCH = 4                    # super-nodes per phase-A chunk
NCH = MC // MCH            # 8 chunks
RCH = MCH * O * K * K      # 5120 kb-rows per chunk
HCH = MCH * O * K          # 640 h-rows per chunk
RH = MC * O * K            # 5120 h-rows per core
RKB = MC * O * K * K       # 40960 kb-rows per core

F32 = mybir.dt.float32
F32R = mybir.dt.float32r
BF16 = mybir.dt.bfloat16
AF = mybir.ActivationFunctionType
ALU = mybir.AluOpType
AX = mybir.AxisListType

DEBUG_DUMPS = bool(int(os.environ.get("KERNEL_DEBUG_DUMPS", "0")))


class PatchedTileContext(tile.TileContext):
    """This container's walrus rejects >1 sem wait per instruction; split the
    final drain's waits onto preceding single-wait NOPs."""
    N_SPARE = 30

    def _drain_and_barrier(self, tick_clock, wait_clock):
        nc = self.nc
        nops = [nc.sync.nop(nofuse=True) for _ in range(self.N_SPARE)]
        drain_inst = nc.sync.drain()
        wait_clock.add_sem_waits(
            drain_inst.ins, ScopedClock({None: tick_clock.global_clock})
        )
        si = drain_inst.ins.sync_info
        waits = list(si.on_wait) if si is not None else []
        if len(waits) > 1:
            assert len(waits) - 1 <= len(nops), f"drain waits: {len(waits)}"
            for i, wt in enumerate(waits[:-1]):
                nops[i].ins.sync_info = mybir.SyncInfo(on_wait=[wt],
                                                       on_update=[])
            drain_inst.ins.sync_info = mybir.SyncInfo(
                on_wait=[waits[-1]],
                on_update=list(si.on_update) if si is not None else [],
            )
        nc.all_engine_barrier()
        popped = nc._tile_sem_poison_stack.pop()
        assert popped is self._sem_poison
        nc.clear_and_free_semaphores(list(self.sems.allocated().values()))
        nc.all_engine_barrier()


# ----------------------------------------------------------------------------
# host-side math
# ----------------------------------------------------------------------------
def _ori_grid(n=O):
    i = np.arange(n, dtype=np.float64)
    theta = (np.pi * i * (1.0 + np.sqrt(5.0))) % (2.0 * np.pi)
    phi = np.arccos(1.0 - 2.0 * (i + 0.5) / (n - 1 + 1.0))
    g = np.stack([np.cos(theta) * np.sin(phi),
                  np.sin(theta) * np.sin(phi),
                  np.cos(phi)], axis=-1)
    return g.astype(np.float32)


def _knn_idx(x):
    xb = x[0].astype(np.float32)
    diff = xb[:, None, :] - xb[None, :, :]
    sq = (diff * diff).astype(np.float32)
    d2 = ((sq[..., 0] + sq[..., 1]) + sq[..., 2]).astype(np.float32)
    return np.argsort(d2, axis=-1, kind="stable")[:, :K]


_C9 = np.zeros((9, 14), dtype=np.float32)
for _j, _fs in enumerate([[0], [1], [2], [3, 4], [5], [6], [7, 8, 10],
                          [9, 11, 12], [13]]):
    for _f in _fs:
        _C9[_j, _f] = 1.0


def _mono9(x, y):
    x2 = x * x
    y2 = y * y
    xy = x * y
    return np.stack([x, y, x2, xy, y2, x2 * x, x2 * y, xy * y, y2 * y], axis=0)


# channel permutation: c = 6j+ct -> PI[c] = BOFF[ct] + j  (block-contiguous)
BOFF = [0, 22, 44, 65, 86, 107]
PI = np.zeros(H, dtype=np.int64)
for _c in range(H):
    PI[_c] = BOFF[_c % 6] + _c // 6
PINV = np.argsort(PI)


def _host_prep(inputs):
    f32 = np.float32
    g = {k: np.asarray(v, f32) for k, v in inputs.items()}
    ori = _ori_grid()

    idx = _knn_idx(g["x"])                                # [N,K]
    xn = g["x"][0][idx]                                   # [N,K,3]
    pn = g["pos"][0][idx]                                 # [N,K,3]
    rel = pn[:, None, :, :] - pn[:, :, None, :]           # [M,a,b,3]
    relsq = (rel * rel).astype(f32)
    relsq = ((relsq[..., 0] + relsq[..., 1]) + relsq[..., 2]).astype(f32)

    rel_s = rel.transpose(3, 0, 1, 2).reshape(3, M * K * K)   # [3,(m,a,b)]
    inv1 = (ori @ rel_s).astype(f32)                      # [O,(m,a,b)]
    t = (1.0 - inv1).astype(f32)
    i2 = ((t * t) * relsq.reshape(1, -1)).astype(f32)
    inv2 = np.sqrt(i2 + np.float32(1e-12)).astype(f32)
    monos = _mono9(inv1, inv2)                            # [9,O,(m,a,b)]
    monos = (monos.reshape(9, O, M, K, K)                 # j,o,m,a,b
             .transpose(0, 2, 1, 4, 3)                    # j,m,o,b,a
             .reshape(9, M * O * K * K)).astype(f32)

    a9 = (_C9 @ g["W1"]).astype(f32)                      # [9,H]
    Wfk_s = (g["Wfk"] / f32(H)).astype(f32)               # [L,BD,H]
    Wb1p = np.stack([g["gamma"][l][:, None] * g["Wb1"][l] for l in range(L)])
    bias1p = np.stack([g["beta"][l] @ g["Wb1"][l] + g["bb1"][l]
                       for l in range(L)])                # [L,4H]
    Wr_s = (g["Wr"] / f32(L * K * O)).astype(f32)         # [L,H,32]

    oi = (ori @ ori.T).astype(f32)
    mono_oi = np.stack([oi, oi * oi, (oi * oi) * oi], 0).reshape(3, O * O)

    xn_cm = xn.reshape(M * K, 3).T.astype(f32)            # [3,(m,s)]

    # cbias per block: cb[(ct*20+o), l*22+j] = cbias[l, 6j+ct]
    cb = np.zeros((120, L * 22), f32)
    for l in range(L):
        for j in range(22):
            nch = min(6, H - 6 * j)
            for ct in range(nch):
                cb[ct * 20 + np.arange(O), l * 22 + j] = \
                    g["cbias"][l, 6 * j + ct]

    # channel permutation folds (see PI): columns that produce channel-
    # indexed outputs, rows that consume channel-indexed inputs
    We_p = np.zeros_like(g["We"]); We_p[:, PI] = g["We"]
    Wk_p = np.zeros_like(g["Wk"]); Wk_p[:, :, PI] = g["Wk"]
    Wfk_p = np.zeros_like(Wfk_s); Wfk_p[:, :, PI] = Wfk_s
    # permute rows so row PI[c] holds old row c
    Wb1p_p = np.zeros_like(Wb1p)
    for l in range(L):
        Wb1p_p[l][PI] = Wb1p[l]
    Wr_p = np.zeros_like(Wr_s)
    for l in range(L):
        Wr_p[l][PI] = Wr_s[l]
    Wb2_p = np.zeros_like(g["Wb2"])
    Wb2_p[:, :, PI] = g["Wb2"]
    bb2_p = np.zeros_like(g["bb2"])
    bb2_p[:, PI] = g["bb2"]

    # Wb2 as [128, 4, 128] k-tiles
    wb2 = [np.ascontiguousarray(
        Wb2_p[l].reshape(4, 128, 128).transpose(1, 0, 2)) for l in range(L)]

    shared = dict(
        a9=a9, b1=g["b1"].reshape(H, 1), W2=g["W2"], b2=g["b2"].reshape(H, 1),
        We=We_p, af=g["Wf1"], bf1=g["bf1"].reshape(H, 1), Wf2=g["Wf2"],
        bf2=g["bf2"].reshape(H, 1), monoi=mono_oi,
        Wk0=Wk_p[0], Wk1=Wk_p[1],
        Wb1p0=Wb1p_p[0], Wb1p1=Wb1p_p[1],
        bias1p0=np.ascontiguousarray(bias1p[0].reshape(4, H).T),  # [128,4]
        bias1p1=np.ascontiguousarray(bias1p[1].reshape(4, H).T),
        Wb2_0=wb2[0], Wb2_1=wb2[1],
        bb2_0=bb2_p[0].reshape(H, 1), bb2_1=bb2_p[1].reshape(H, 1),
        Wr0=Wr_p[0], Wr1=Wr_p[1],
        Wfk0=Wfk_p[0], Wfk1=Wfk_p[1],
        cb=cb, oriT=ori, ident=np.eye(128, dtype=f32),
    )

    per_core = []
    monos_pc = monos.reshape(9, M, O * K * K)
    xn_pc = xn_cm.reshape(3, M, K)
    for c in range(N_CORES):
        ms = slice(c * MC, (c + 1) * MC)
        per_core.append(dict(
            monos=np.ascontiguousarray(monos_pc[:, ms].reshape(9, RKB)),
            xn=np.ascontiguousarray(xn_pc[:, ms].reshape(3, MC * K)),
        ))
    return shared, per_core, ori, g


# ----------------------------------------------------------------------------
# device program
# ----------------------------------------------------------------------------
def _r(ap):
    return ap.bitcast(F32R)


_DW_SHAPES = [
    ("a9", [9, H]), ("b1", [H, 1]), ("W2", [H, BD]), ("b2", [H, 1]),
    ("We", [3, H]), ("af", [3, H]), ("bf1", [H, 1]), ("Wf2", [H, BD]),
    ("bf2", [H, 1]), ("monoi", [3, O * O]),
    ("Wk0", [BD, H]), ("Wk1", [BD, H]),
    ("Wb1p0", [H, 4 * H]), ("Wb1p1", [H, 4 * H]),
    ("bias1p0", [H, 4]), ("bias1p1", [H, 4]),
    ("Wb2_0", [128, 4, 128]), ("Wb2_1", [128, 4, 128]),
    ("bb2_0", [H, 1]), ("bb2_1", [H, 1]),
    ("Wr0", [H, 32]), ("Wr1", [H, 32]),
    ("Wfk0", [BD, H]), ("Wfk1", [BD, H]),
    ("cb", [120, L * 22]), ("oriT", [O, 3]), ("ident", [128, 128]),
]


_F32R_W = {"a9", "W2", "We", "af", "Wf2", "monoi", "Wb1p0", "Wb1p1",
           "Wr0", "Wr1", "Wfk0", "Wfk1", "oriT", "ident"}


def _build_program():
    nc = bass.Bass()

    d_monos = nc.dram_tensor("monos", [9, RKB], F32R, kind="ExternalInput")
    d_xn = nc.dram_tensor("xn", [3, MC * K], F32R, kind="ExternalInput")
    dw = {}
    for name, shape in _DW_SHAPES:
        dt_ = F32R if name in _F32R_W else F32
        dw[name] = nc.dram_tensor(name, shape, dt_, kind="ExternalInput")

    d_scal = nc.dram_tensor("scal", [OUT, MC], F32, kind="ExternalOutput")
    d_vec = nc.dram_tensor("vec", [3, MC * OV], F32, kind="ExternalOutput")
    d_spill = nc.dram_tensor("kb_spill", [128, RKB], BF16, kind="Internal")
    d_bx1 = nc.dram_tensor("bx1", [6, 22, O, MC * K], BF16, kind="Internal")
    d_bx2 = nc.dram_tensor("bx2", [6, 22, O, MC * K], BF16, kind="Internal")
    d_fk = nc.dram_tensor("fk_bounce", [L, 128, O * O], BF16,
                          kind="Internal")

    dbg = {}
    if DEBUG_DUMPS:
        for name, shape, dt_ in [
            ("dbg_kb", [128, RCH], BF16), ("dbg_kern", [128, RCH], BF16),
            ("dbg_x1_0", [128, RH], BF16), ("dbg_x2s_0", [128, RH], BF16),
            ("dbg_z_0", [128, RH], F32), ("dbg_h1", [128, RH], F32),
            ("dbg_rsum", [32, RH], F32), ("dbg_hL0", [128, RH], F32),
            ("dbg_x1_1", [128, RH], BF16), ("dbg_x2s_1", [128, RH], BF16),
            ("dbg_z_1", [128, RH], F32), ("dbg_y1", [128, 4 * 1024], BF16),
        ]:
            dbg[name] = nc.dram_tensor(name, shape, dt_,
                                       kind="ExternalOutput")

    import os as _os
    _lin = bool(int(_os.environ.get("KERNEL_LINEARIZE", "0")))
    with PatchedTileContext(nc, linearize=_lin) as tc:
        _emit(tc, nc, d_monos, d_xn, dw, d_scal, d_vec, d_spill,
              (d_bx1, d_bx2, d_fk), dbg)
    _install_multiwait_split(nc)
    return nc


def _split_multiwait_json(data):
    """Walrus in this container rejects >1 sem wait per instruction; move
    extra waits onto preceding same-engine NoOps at the BIR JSON level."""
    import json
    d = json.loads(data)
    n_split = 0
    for fn in d["functions"]:
        for blk in fn["blocks"]:
            out = []
            for inst in blk["instructions"]:
                si = inst.get("sync_info")
                waits = (si or {}).get("on_wait") or []
                if len(waits) > 1:
                    n_split += 1
                    for i, wt in enumerate(waits[:-1]):
                        out.append({
                            "name": inst["name"] + f"__w{i}",
                            "opcode": "NoOp",
                            "engine": inst["engine"],
                            "ins": [], "outs": [],
                            "sync_info": {"on_wait": [wt], "on_update": []},
                        })
                    si["on_wait"] = [waits[-1]]
                out.append(inst)
            blk["instructions"] = out
    return json.dumps(d).encode()


def _install_multiwait_split(nc):
    orig = nc.to_json_bytes

    def patched(*a, **k):
        return _split_multiwait_json(orig(*a, **k))

    nc.to_json_bytes = patched


def _emit(tc, nc, d_monos, d_xn, dw, d_scal, d_vec, d_spill, bounce, dbg):
    d_bx1, d_bx2, d_fk = bounce
    ctx = contextlib.ExitStack()
    with ctx:
        wpool = ctx.enter_context(tc.tile_pool(name="weights", bufs=1))
        live = ctx.enter_context(tc.tile_pool(name="live", bufs=1))

        w = {}
        for name in ["a9", "b1", "W2", "b2", "We", "af", "bf1", "Wf2", "bf2",
                     "monoi", "Wb1p0", "Wb1p1", "bias1p0", "bias1p1",
                     "bb2_0", "bb2_1", "Wr0", "Wr1", "Wfk0", "Wfk1",
                     "cb", "oriT", "ident"]:
            t = wpool.tile(list(dw[name].shape),
                           F32R if name in _F32R_W else F32, tag=name)
            nc.sync.dma_start(t[:], dw[name][:])
            w[name] = t
        wb = {}
        for name in ["Wk0", "Wk1", "Wb2_0", "Wb2_1"]:
            tf = wpool.tile(list(dw[name].shape), F32, tag=name + "_f")
            nc.sync.dma_start(tf[:], dw[name][:])
            t = wpool.tile(list(dw[name].shape), BF16, tag=name)
            nc.vector.tensor_copy(t[:], tf[:])
            wb[name] = t
        ones_b = wpool.tile([128, 1], BF16, tag="ones_b")
        nc.vector.memset(ones_b[:], 1.0)
        ones_f = wpool.tile([128, 1], F32, tag="ones_f")
        nc.vector.memset(ones_f[:], 1.0)
        eps_t = wpool.tile([128, 1], F32, tag="eps_t")
        nc.vector.memset(eps_t[:], 1e-5)
        ident_b = wpool.tile([128, 128], BF16, tag="ident_b")
        nc.vector.tensor_copy(ident_b[:], w["ident"][:])
        cst = dict(ones_b=ones_b, ones_f=ones_f, eps_t=eps_t,
                   ident_b=ident_b)

        h = live.tile([128, RH], F32R, tag="h")             # rows (o,m,s)
        x1 = live.tile([128, RH], BF16, tag="x1")           # rows (o,m,a)
        rsum = live.tile([32, RH], F32, tag="rsum")         # rows (p,m,a)
        h0emb = live.tile([128, MC * K], F32R, tag="h0emb")  # (m,s)
        xn_t = live.tile([3, MC * K], F32R, tag="xn")
        nc.sync.dma_start(xn_t[:], d_xn[:])
        fkb = [live.tile([120, 22, 120], BF16, tag=f"fkb{l}",
                         name=f"fkb{l}") for l in range(L)]

        # ---- setup: h0emb + fiber basis ----
        with tc.tile_pool(name="ps_set", bufs=2, space="PSUM") as pss, \
             tc.tile_pool(name="sb_set", bufs=2) as sbs:
            pt = pss.tile([128, MC * K], F32, tag="pA")
            nc.tensor.matmul(pt[:], (w["We"][:]), (xn_t[:]),
                             start=True, stop=True)
            nc.scalar.activation(h0emb[:], pt[:], AF.Copy)
            # materialize h0 broadcast over fiber: h[c,(o,m,s)] = h0emb[c,(m,s)]
            hdst = h[:].rearrange("c (o ms) -> c o ms", o=O)
            hsrc = bass.AP(tensor=h0emb.tensor, offset=h0emb.offset,
                           ap=[list(h0emb.ap[0]), [0, O], [1, MC * K]])
            nc.sync.dma_start(hdst, hsrc)

            uf = pss.tile([128, O * O], F32, tag="pA")
            nc.tensor.matmul(uf[:], (w["af"][:]), (w["monoi"][:]),
                             start=True, stop=True)
            s1f = sbs.tile([128, O * O], F32R, tag="s1f")
            nc.scalar.activation(s1f[:], uf[:], AF.Gelu, bias=w["bf1"][:])
            kbf = pss.tile([128, O * O], F32, tag="pA")
            nc.tensor.matmul(kbf[:], (w["Wf2"][:]), _r(s1f[:]),
                             start=True, stop=True)
            fbv = sbs.tile([128, O * O], F32R, tag="fbv")
            nc.scalar.activation(fbv[:], kbf[:], AF.Gelu, bias=w["bf2"][:])
            for l in range(L):
                fkp = pss.tile([128, O * O], F32, tag="pA")
                nc.tensor.matmul(fkp[:], (w[f"Wfk{l}"][:]), _r(fbv[:]),
                                 start=True, stop=True)
                fk_s = sbs.tile([128, O * O], BF16, tag="fk_s")
                nc.vector.tensor_copy(fk_s[:], fkp[:])
                nc.vector.memset(fkb[l][:], 0.0)
                # bounce fk through DRAM, then block-diagonal reads.
                # fk rows are PI-permuted channels; fk symmetric in (p,o).
                nc.sync.dma_start(d_fk[l], fk_s[:])
                for ct in range(6):
                    nblk = 22 if ct < 2 else 21
                    dst = fkb[l][ct * O:(ct + 1) * O, 0:nblk,
                                 ct * O:(ct + 1) * O]         # (o, j, p)
                    src = d_fk[l, BOFF[ct]:BOFF[ct] + nblk, :].rearrange(
                        "j (o p) -> o j p", o=O)
                    nc.sync.dma_start(dst, src)

        # ---- layers ----
        for l in range(L):
            _phase_a(tc, nc, l, d_monos, d_spill, w, wb, h, h0emb, x1, dbg)
            if DEBUG_DUMPS and l == 1:
                nc.sync.dma_start(dbg["dbg_x1_1"][:], x1[:])
            _phase_b(tc, nc, l, w, wb, cst, fkb[l], h, h0emb, x1, rsum,
                     (d_bx1, d_bx2), dbg)
            if DEBUG_DUMPS and l == 0:
                nc.sync.dma_start(dbg["dbg_hL0"][:], h[:].bitcast(F32))

        # ---- pooling ----
        with tc.tile_pool(name="ps_pool", bufs=2, space="PSUM") as psp, \
             tc.tile_pool(name="sb_pool", bufs=2) as sbp:
            # step 1: reduce slots -> pa [32, (p, m)]
            pa = sbp.tile([32, O * MC], F32R, tag="pa")
            rs_pm = rsum[:, :].rearrange("ch (pm g) -> ch pm g", g=K)
            with nc.allow_low_precision(reason="f32r is full fp32 storage"):
                nc.vector.tensor_reduce(pa[:], rs_pm, AX.X, ALU.add)
            # step 2 (scaler): reduce over p with (m, p) iteration
            scal = sbp.tile([OUT, MC], F32, tag="scal")
            pa_sc = bass.AP(tensor=pa.tensor, offset=pa.offset,
                            ap=[[pa.ap[0][0], OUT], [1, MC], [MC, O]])
            nc.vector.tensor_reduce(scal[:], pa_sc, AX.X, ALU.add)
            nc.sync.dma_start(d_scal[:], scal[:])

            # vat [20, (v, m)]
            vat = sbp.tile([O, OV * MC], F32R, tag="vat")
            for v in range(OV):
                dst = vat[:, v * MC:(v + 1) * MC]
                src = pa[OUT + v:OUT + v + 1, :].rearrange(
                    "c (p m) -> c p m", p=O)
                nc.sync.dma_start(dst, src)
            pv = psp.tile([3, MC * OV], F32, tag="pv")
            nc.tensor.matmul(pv[:], (w["oriT"][:]), (vat[:]),
                             start=True, stop=True)
            vec_o = sbp.tile([3, MC * OV], F32, tag="vec_o")
            nc.scalar.activation(vec_o[:], pv[:], AF.Copy)
            nc.sync.dma_start(d_vec[:], vec_o[:])

        if DEBUG_DUMPS:
            nc.sync.dma_start(dbg["dbg_rsum"][:], rsum[:])
            nc.sync.dma_start(dbg["dbg_h1"][:], h[:].bitcast(F32))


def _phase_a(tc, nc, l, d_monos, d_spill, w, wb, h, h0emb, x1, dbg):
    """Per chunk: (l==0) monos -> mm1 -> gelu -> mm2 -> gelu -> kb bf16
    (spilled to HBM); (l==1) reload kb.  Then kern = kb @ Wk_l and the
    K-neighbor einsum accumulating x1."""
    SUB = 1024
    NSUB = RCH // SUB  # 5
    with tc.tile_pool(name="apool", bufs=2) as apool, \
         tc.tile_pool(name="apool1", bufs=1) as apool1, \
         tc.tile_pool(name="pwpool", bufs=2) as pwpool, \
         tc.tile_pool(name="ps_mm", bufs=3, space="PSUM") as psmm, \
         tc.tile_pool(name="ps_kern", bufs=2, space="PSUM") as pskern:
        for ch in range(NCH):
            r0 = ch * RCH
            kbt = apool.tile([128, RCH], BF16, tag="kbt")
            if l == 0:
                mt = apool1.tile([9, RCH], F32R, tag="mt")
                nc.sync.dma_start(mt[:], d_monos[:, r0:r0 + RCH])
                for sb in range(NSUB):
                    s0 = sb * SUB
                    u_ps = psmm.tile([128, SUB], F32, tag="mmps")
                    for q in range(2):
                        nc.tensor.matmul(
                            u_ps[:, q * 512:(q + 1) * 512],
                            (w["a9"][:]),
                            (mt[:, s0 + q * 512:s0 + (q + 1) * 512]),
                            start=True, stop=True)
                    s1t = apool.tile([128, SUB], F32R, tag="s1t")
                    nc.scalar.activation(s1t[:], u_ps[:], AF.Gelu,
                                         bias=w["b1"][:])
                    k_ps = psmm.tile([128, SUB], F32, tag="mmps")
                    for q in range(2):
                        nc.tensor.matmul(
                            k_ps[:, q * 512:(q + 1) * 512],
                            (w["W2"][:]),
                            (s1t[:, q * 512:(q + 1) * 512]),
                            start=True, stop=True)
                    nc.scalar.activation(kbt[:, s0:s0 + SUB], k_ps[:],
                                         AF.Gelu, bias=w["b2"][:])
                nc.sync.dma_start(d_spill[:, r0:r0 + RCH], kbt[:])
            else:
                nc.sync.dma_start(kbt[:], d_spill[:, r0:r0 + RCH])

            kernt = apool.tile([128, RCH], BF16, tag="kernt")
            for sb in range(RCH // 512):
                s0 = sb * 512
                kp = pskern.tile([128, 512], F32, tag="kernps")
                nc.tensor.matmul(kp[:], wb[f"Wk{l}"][:],
                                 kbt[:, s0:s0 + 512], start=True, stop=True)
                nc.vector.tensor_copy(kernt[:, s0:s0 + 512], kp[:])

            if DEBUG_DUMPS and l == 0 and ch == 0:
                nc.sync.dma_start(dbg["dbg_kb"][:], kbt[:])
                nc.sync.dma_start(dbg["dbg_kern"][:], kernt[:])

            # x1[c,(o,m,a)] = sum_b kern[c,(m,o,b,a)] * h[c,(o,m,b)]
            kern5 = kernt[:].rearrange("c (m o b a) -> c m o b a",
                                       m=MCH, o=O, b=K)
            # h view (m, o, s): h rows are (o, m, s) core-wide
            h4 = h[:].rearrange("c (o m s) -> c o m s", o=O, m=MC)
            pw = []
            for b in range(K):
                pwt = pwpool.tile([128, HCH], BF16, tag=f"pw{b}")
                hsl = bass.AP(
                    tensor=h.tensor, offset=h.offset + b + ch * MCH * K,
                    ap=[list(h.ap[0]), [K, MCH], [MC * K, O], [0, K]])
                pwv = pwt[:].rearrange("c (m o a) -> c m o a", m=MCH, o=O)
                nc.vector.tensor_tensor(pwv, kern5[:, :, :, b, :], hsl,
                                        ALU.mult)
                pw.append(pwt)
            nc.vector.tensor_tensor(pw[0][:], pw[0][:], pw[1][:], ALU.add)
            nc.vector.tensor_tensor(pw[2][:], pw[2][:], pw[3][:], ALU.add)
            nc.vector.tensor_tensor(pw[4][:], pw[4][:], pw[5][:], ALU.add)
            nc.vector.tensor_tensor(pw[6][:], pw[6][:], pw[7][:], ALU.add)
            nc.vector.tensor_tensor(pw[0][:], pw[0][:], pw[2][:], ALU.add)
            nc.vector.tensor_tensor(pw[4][:], pw[4][:], pw[6][:], ALU.add)
            x1o = bass.AP(
                tensor=x1.tensor, offset=x1.offset + ch * MCH * K,
                ap=[list(x1.ap[0]), [K, MCH], [MC * K, O], [1, K]])
            pwv0 = pw[0][:].rearrange("c (m o a) -> c m o a", m=MCH, o=O)
            pwv4 = pw[4][:].rearrange("c (m o a) -> c m o a", m=MCH, o=O)
            nc.vector.tensor_tensor(x1o, pwv0, pwv4, ALU.add)


def _phase_b(tc, nc, l, w, wb, cst, fkb_l, h, h0emb, x1, rsum, bounce2,
             dbg):
    d_bx1, d_bx2 = bounce2
    """Fiber conv (block-diag PE), LayerNorm (PE transposes + deferred
    per-row scalars), bottleneck MLP, residual update, readout."""
    with tc.tile_pool(name="bpool", bufs=1) as bpool, \
         tc.tile_pool(name="bpool2", bufs=2) as bpool2:
        # ---- shuffle x1 -> x1J blocks ----
        x1j = bpool.tile([120, 22, MC * K], BF16, tag="x1j")
        for ct in range(6):
            nblk = 22 if ct < 2 else 21
            src = x1[BOFF[ct]:BOFF[ct] + nblk, :].rearrange(
                "j (o ma) -> j o ma", o=O)
            nc.scalar.dma_start(d_bx1[ct, 0:nblk], src)
        for ct in range(6):
            nblk = 22 if ct < 2 else 21
            dst = x1j[ct * O:(ct + 1) * O, 0:nblk, :]
            src = d_bx1[ct, 0:nblk].rearrange("j o ma -> o j ma")
            nc.scalar.dma_start(dst, src)

        # ---- fiber conv ----
        x2e = bpool.tile([120, 22, MC * K], BF16, tag="x2e")
        with tc.tile_pool(name="ps_fib", bufs=2, space="PSUM") as psf:
            for j in range(22):
                fp = psf.tile([120, MC * K], F32, tag="fib")
                nc.tensor.matmul(fp[:], fkb_l[:, j, :], x1j[:, j, :],
                                 start=True, stop=True)
                nc.vector.tensor_scalar_add(
                    x2e[:, j, :], fp[:],
                    w["cb"][:, l * 22 + j:l * 22 + j + 1])

        # ---- shuffle back -> x2s [c,(m,p,a)] ----
        x2s = bpool.tile([128, RH], BF16, tag="x2s")
        for ct in range(6):
            nblk = 22 if ct < 2 else 21
            src = x2e[ct * O:(ct + 1) * O, 0:nblk, :]   # (p, j, ma)
            dst = d_bx2[ct, 0:nblk].rearrange("j p ma -> p j ma")
            nc.scalar.dma_start(dst, src)
        for ct in range(6):
            nblk = 22 if ct < 2 else 21
            dst = x2s[BOFF[ct]:BOFF[ct] + nblk, :].rearrange(
                "j (p ma) -> j p ma", p=O)
            src = d_bx2[ct, 0:nblk]
            nc.scalar.dma_start(dst, src)

        if DEBUG_DUMPS and l == 0:
            nc.sync.dma_start(dbg["dbg_x1_0"][:], x1[:])
        if DEBUG_DUMPS:
            nc.sync.dma_start(dbg[f"dbg_x2s_{l}"][:], x2s[:])

        # ---- LayerNorm -> z [c,(m,p,a)] f32 ----
        z = bpool.tile([128, RH], F32R, tag="z")
        with tc.tile_pool(name="ps_ln", bufs=2, space="PSUM") as psl, \
             tc.tile_pool(name="sb_ln", bufs=2) as sbl:
            for g512 in range(RH // 512):
                s0 = g512 * 512
                x2sq = sbl.tile([128, 512], F32, tag="x2sq")
                nc.vector.tensor_tensor(x2sq[:], x2s[:, s0:s0 + 512],
                                        x2s[:, s0:s0 + 512], ALU.mult)
                s1p = psl.tile([128, 4], F32, tag="s1p")
                s2p = psl.tile([128, 4], F32, tag="s2p")
                for gi in range(4):
                    c0 = s0 + gi * 128
                    nc.tensor.matmul(s1p[:, gi:gi + 1],
                                     x2s[:, c0:c0 + 128], cst["ones_b"][:],
                                     start=True, stop=True)
                    nc.tensor.matmul(s2p[:, gi:gi + 1],
                                     (x2sq[:, gi * 128:(gi + 1) * 128]),
                                     (cst["ones_f"][:]),
                                     start=True, stop=True)
                muT = sbl.tile([128, 4], F32, tag="muT")
                nc.vector.tensor_scalar_mul(muT[:], s1p[:], 1.0 / 128.0)
                musq = sbl.tile([128, 4], F32, tag="musq")
                nc.vector.tensor_tensor(musq[:], muT[:], muT[:], ALU.mult)
                varT = sbl.tile([128, 4], F32, tag="varT")
                nc.vector.scalar_tensor_tensor(
                    out=varT[:], in0=s2p[:], scalar=1.0 / 128.0,
                    in1=musq[:], op0=ALU.mult, op1=ALU.subtract)
                sdT = sbl.tile([128, 4], F32, tag="sdT")
                nc.scalar.activation(sdT[:], varT[:], AF.Sqrt,
                                     bias=cst["eps_t"][:])
                rstdT = sbl.tile([128, 4], F32, tag="rstdT")
                nc.vector.reciprocal(rstdT[:], sdT[:])
                for gi in range(4):
                    c0 = s0 + gi * 128
                    xtp = psl.tile([128, 128], BF16, tag="xtp")
                    nc.tensor.transpose(xtp[:], x2s[:, c0:c0 + 128],
                                        cst["ident_b"][:])
                    zT = sbl.tile([128, 128], F32R, tag="zT")
                    nc.vector.tensor_scalar(
                        out=zT[:], in0=xtp[:],
                        scalar1=muT[:, gi:gi + 1],
                        scalar2=rstdT[:, gi:gi + 1],
                        op0=ALU.subtract, op1=ALU.mult)
                    zbp = psl.tile([128, 128], F32R, tag="zbp")
                    nc.tensor.transpose(zbp[:], zT[:], w["ident"][:])
                    nc.vector.tensor_copy(z[:, c0:c0 + 128], zbp[:])

        if DEBUG_DUMPS:
            nc.sync.dma_start(dbg[f"dbg_z_{l}"][:], z[:].bitcast(F32))

        # ---- bottleneck + h update + readout ----
        RC = 1024
        with tc.tile_pool(name="ps_bt", bufs=2, space="PSUM") as psb, \
             tc.tile_pool(name="ps_ro", bufs=2, space="PSUM") as psr:
            for rc in range(RH // RC):
                r0 = rc * RC
                y1 = bpool2.tile([128, 4, RC], BF16, tag="y1")
                for j in range(4):
                    yp = psb.tile([128, RC], F32, tag="y1p")
                    for q in range(2):
                        nc.tensor.matmul(
                            yp[:, q * 512:(q + 1) * 512],
                            (w[f"Wb1p{l}"][:, j * 128:(j + 1) * 128]),
                            (z[:, r0 + q * 512:r0 + (q + 1) * 512]),
                            start=True, stop=True)
                    nc.scalar.activation(
                        y1[:, j, :], yp[:], AF.Gelu,
                        bias=w[f"bias1p{l}"][:, j:j + 1])
                if DEBUG_DUMPS and l == 0 and rc == 0:
                    nc.sync.dma_start(dbg["dbg_y1"][:],
                                      y1[:].rearrange("c j r -> c (j r)"))
                for q in range(2):
                    q0 = r0 + q * 512
                    yo = psb.tile([128, 512], F32, tag="yo")
                    for kt in range(4):
                        nc.tensor.matmul(
                            yo[:], wb[f"Wb2_{l}"][:, kt, :],
                            y1[:, kt, q * 512:(q + 1) * 512],
                            start=(kt == 0), stop=(kt == 3))
                    nc.vector.scalar_tensor_tensor(
                        out=h[:, q0:q0 + 512], in0=yo[:],
                        scalar=w[f"bb2_{l}"][:],
                        in1=h[:, q0:q0 + 512],
                        op0=ALU.add, op1=ALU.add)
                    ro = psr.tile([32, 512], F32, tag="ro")
                    nc.tensor.matmul(ro[:], (w[f"Wr{l}"][:]),
                                     (h[:, q0:q0 + 512]),
                                     start=True, stop=True)
                    if l == 0:
                        nc.vector.tensor_copy(rsum[:, q0:q0 + 512], ro[:])
                    else:
                        nc.vector.tensor_tensor(rsum[:, q0:q0 + 512],
                                                rsum[:, q0:q0 + 512],
                                                ro[:], ALU.add)


# ----------------------------------------------------------------------------
# public entry
# ----------------------------------------------------------------------------
_CACHED = {}

_SHARED_KEYS = [name for name, _ in _DW_SHAPES]


def kernel(**inputs):
    shared, per_core, ori, g = _host_prep(inputs)
    if "nc" not in _CACHED:
        _CACHED["nc"] = _build_program()
    nc = _CACHED["nc"]

    shared_arrs = {kname: np.ascontiguousarray(shared[kname],
                                               dtype=np.float32)
                   for kname in _SHARED_KEYS}
    in_maps = []
    for c in range(N_CORES):
        mmap = dict(shared_arrs)
        mmap["monos"] = per_core[c]["monos"]
        mmap["xn"] = per_core[c]["xn"]
        in_maps.append(mmap)

    res = run_bass_kernel_spmd(nc, in_maps, core_ids=list(range(N_CORES)))

    out_scaler = np.zeros((M, OUT), np.float32)
    out_vector = np.zeros((M, OV, 3), np.float32)
    br = g["br"]
    s_off = br[:, :OUT].sum(0) / np.float32(L)
    v_off = (br[:, OUT:].sum(0))[:, None] * ori.sum(0)[None, :] / np.float32(L)
    for c in range(N_CORES):
        r = res.results[c]
        out_scaler[c * MC:(c + 1) * MC] = r["scal"].T + s_off
        out_vector[c * MC:(c + 1) * MC] = (
            r["vec"].reshape(3, OV, MC).transpose(2, 1, 0) + v_off[None])
    return out_scaler, out_vector
